# revision 1
# baseline (speedup 1.0000x reference)
"""CrossAttention (3x 3D-conv projections + channel attention + residual)
on 8 Trainium2 NeuronCores, data-parallel over batch (2 batches/core).

Reference computation (B=16, C=1024, D=H=W=8, N=D*H*W=512):
  q = conv3d(x, wq, bq); k = conv3d(y, wk, bk); v = conv3d(y, wv, bv)
  scores[b,n,m] = sum_c q[b,c,n] k[b,c,m]; attn = softmax(scores, -1)
  out[b,c,n] = sum_m attn[b,n,m] v[b,c,m];  return out + x

Per-core kernel (all matmuls fp32r: full PE rate at free-dim 512,
~1.5e-4 component relative error):
  - convs: for each kernel tap t (27) and input-channel chunk ic (8),
    a [K=128 x M=128] weight tile against a [K=128 x N=512] shifted slab
    of the zero-padded activation (nested strided AP), accumulated in
    PSUM over all 216 (t, ic) steps; both batches + 4 output chunks
    share each weight-tile DMA (8 PSUM banks live).
  - q/k/vT are spilled to DRAM scratch after each conv (SBUF can't hold
    the padded activations and all projection outputs at once) and
    reloaded per batch for the attention phase.
  - v is transposed on the TensorEngine (128x128 identity-matmul blocks)
    to give vT[m, c] for the second attention matmul.
  - bq/bk are added during PSUM evacuation; bv is folded into the
    residual (softmax rows sum to 1, so attn @ (v + bv) = attn @ v + bv).
"""
import sys

sys.path.insert(0, '/opt/trn_rl_repo')

import numpy as np

from concourse import bacc, mybir, masks
from concourse.tile import TileContext
from concourse.bass_utils import run_bass_kernel_spmd

F32 = mybir.dt.float32
F32R = mybir.dt.float32r

B, C, N = 16, 1024, 512
NCORES = 8
BPC = B // NCORES          # batches per core
ICH = C // 128             # input/output channel chunks of 128
PAD = 1000                 # 10*10*10 padded volume

_CACHED_NC = None
LAST_RESULTS = None


def _taps():
    for t in range(27):
        yield t, t // 9, (t // 3) % 3, t % 3


def _build():
    nc = bacc.Bacc("TRN2", target_bir_lowering=False, debug=False)

    xp = nc.dram_tensor("xp", [BPC, C, PAD], F32R, kind="ExternalInput")
    yp = nc.dram_tensor("yp", [BPC, C, PAD], F32R, kind="ExternalInput")
    xres = nc.dram_tensor("xres", [BPC, C, N], F32, kind="ExternalInput")
    wqp = nc.dram_tensor("wqp", [27, C, C], F32R, kind="ExternalInput")
    wkp = nc.dram_tensor("wkp", [27, C, C], F32R, kind="ExternalInput")
    wvp = nc.dram_tensor("wvp", [27, C, C], F32R, kind="ExternalInput")
    bqp = nc.dram_tensor("bqp", [128, ICH], F32, kind="ExternalInput")
    bkp = nc.dram_tensor("bkp", [128, ICH], F32, kind="ExternalInput")
    out = nc.dram_tensor("out", [BPC, C, N], F32, kind="ExternalOutput")

    # DRAM scratch for projection outputs between conv and attention
    q_dr = nc.dram_tensor("q_dr", [BPC, 128, ICH, N], F32R)
    k_dr = nc.dram_tensor("k_dr", [BPC, 128, ICH, N], F32R)
    vT_dr = nc.dram_tensor("vT_dr", [BPC, 128, 4, C], F32R)

    with TileContext(nc) as tc:
        with tc.tile_pool(name="const", bufs=1) as cpool, \
             tc.tile_pool(name="psum", bufs=1, space="PSUM") as psp:

            ident = cpool.tile([128, 128], F32, tag="ident")
            masks.make_identity(nc, ident[:])
            bq_t = cpool.tile([128, ICH], F32, tag="bq_t")
            nc.sync.dma_start(bq_t[:], bqp[:])
            bk_t = cpool.tile([128, ICH], F32, tag="bk_t")
            nc.sync.dma_start(bk_t[:], bkp[:])

            def psum_tile(i):
                return psp.tile([128, 512], F32, tag=f"ps{i}", name=f"ps{i}")

            with tc.tile_pool(name="acts", bufs=1) as apool, \
                 tc.tile_pool(name="wts", bufs=8) as wpool, \
                 tc.tile_pool(name="ev", bufs=4) as evpool:

                # padded activations: [128, ic 8, 10, 10, 10]
                def load_pad(src, name):
                    tiles = []
                    for b in range(BPC):
                        t = apool.tile([128, ICH, 10, 10, 10], F32R,
                                       tag=f"{name}{b}", name=f"{name}{b}")
                        nc.sync.dma_start(
                            t[:].rearrange("p i a b c -> p i (a b c)"),
                            src[b].rearrange("(i p) n -> p i n", p=128))
                        tiles.append(t)
                    return tiles

                xpad = load_pad(xp, "xpad")
                ypad = load_pad(yp, "ypad")

                def conv(pads, w_dram, bias_t, dst_dr, transposed):
                    """Accumulate the 27x8-step conv in PSUM, then evacuate
                    to DRAM scratch (with bias, or TensorE-transposed)."""
                    for oh in range(2):
                        pq = [psum_tile(i) for i in range(8)]
                        for t, kd, kh, kw in _taps():
                            for ic in range(ICH):
                                wt = wpool.tile([128, 512], F32R, tag="wt",
                                                name="wt")
                                nc.sync.dma_start(
                                    wt[:],
                                    w_dram[t, ic * 128:(ic + 1) * 128,
                                           oh * 512:(oh + 1) * 512])
                                first = (t == 0 and ic == 0)
                                last = (t == 26 and ic == ICH - 1)
                                for ol in range(4):
                                    lhsT = wt[:, ol * 128:(ol + 1) * 128]
                                    for b in range(BPC):
                                        slab = pads[b][:, ic, kd:kd + 8,
                                                       kh:kh + 8, kw:kw + 8]
                                        nc.tensor.matmul(
                                            pq[ol * BPC + b][:], lhsT, slab,
                                            start=first, stop=last)
                        for ol in range(4):
                            oc = oh * 4 + ol
                            for b in range(BPC):
                                g = ol * BPC + b
                                if not transposed:
                                    stage = evpool.tile([128, 512], F32R,
                                                        tag="stage",
                                                        name="stage")
                                    nc.vector.tensor_scalar_add(
                                        stage[:], pq[g][:],
                                        bias_t[:, oc:oc + 1])
                                    nc.sync.dma_start(dst_dr[b][:, oc, :],
                                                      stage[:])
                                else:
                                    vtmp = evpool.tile([128, 512], F32,
                                                       tag="vtmp", bufs=2,
                                                       name="vtmp")
                                    nc.vector.tensor_copy(vtmp[:], pq[g][:])
                                    ptv = psum_tile(g)
                                    for mc in range(4):
                                        nc.tensor.transpose(
                                            ptv[:, mc * 128:(mc + 1) * 128],
                                            vtmp[:, mc * 128:(mc + 1) * 128],
                                            ident[:])
                                    stage2 = evpool.tile([128, 4, 128], F32R,
                                                         tag="stage2", bufs=2,
                                                         name="stage2")
                                    nc.vector.tensor_copy(
                                        stage2[:],
                                        ptv[:].rearrange("p (m c) -> p m c",
                                                         m=4))
                                    nc.sync.dma_start(
                                        vT_dr[b][:, :,
                                                 oc * 128:(oc + 1) * 128],
                                        stage2[:])

                conv(xpad, wqp, bq_t, q_dr, False)
                conv(ypad, wkp, bk_t, k_dr, False)
                conv(ypad, wvp, None, vT_dr, True)

            # ---------------- attention + residual ----------------
            with tc.tile_pool(name="attn", bufs=1) as dpool, \
                 tc.tile_pool(name="ot", bufs=4) as opool:
                for b in range(BPC):
                    q_t = dpool.tile([128, ICH, N], F32R, tag=f"q_t{b}",
                                     name=f"q_t{b}")
                    nc.sync.dma_start(q_t[:], q_dr[b])
                    k_t = dpool.tile([128, ICH, N], F32R, tag=f"k_t{b}",
                                     name=f"k_t{b}")
                    nc.sync.dma_start(k_t[:], k_dr[b])
                    vT_t = dpool.tile([128, 4, C], F32R, tag=f"vT_t{b}",
                                      name=f"vT_t{b}")
                    nc.sync.dma_start(vT_t[:], vT_dr[b])
                    xr = dpool.tile([128, ICH, N], F32, tag=f"xr{b}",
                                    name=f"xr{b}")
                    nc.sync.dma_start(
                        xr[:], xres[b].rearrange("(i p) n -> p i n", p=128))

                    # scores + softmax, 4 token chunks of 128 rows
                    stats = dpool.tile([128, 3, 4], F32, tag="stats",
                                       name="stats")
                    attn_n = dpool.tile([128, 4, N], F32, tag="attn_n",
                                        name="attn_n")
                    for g in range(4):
                        ps = psum_tile(g)
                        for oc in range(ICH):
                            nc.tensor.matmul(
                                ps[:],
                                q_t[:, oc, g * 128:(g + 1) * 128],
                                k_t[:, oc, :],
                                start=(oc == 0), stop=(oc == ICH - 1))
                        negmax = stats[:, 0, g:g + 1]
                        esum = stats[:, 1, g:g + 1]
                        rinv = stats[:, 2, g:g + 1]
                        nc.vector.reduce_max(negmax, ps[:],
                                             axis=mybir.AxisListType.X,
                                             negate=True)
                        nc.scalar.activation(attn_n[:, g, :], ps[:],
                                             mybir.ActivationFunctionType.Exp,
                                             bias=negmax, accum_out=esum)
                        nc.vector.reciprocal(rinv, esum)
                        nc.vector.tensor_scalar_mul(attn_n[:, g, :],
                                                    attn_n[:, g, :], rinv)
                    # transpose attn -> attnT [m-part, mc, n]
                    attnT = dpool.tile([128, 4, N], F32R, tag="attnT",
                                       name="attnT")
                    for mc in range(4):
                        pt = psum_tile(4 + mc)
                        for g in range(4):
                            nc.tensor.transpose(
                                pt[:, g * 128:(g + 1) * 128],
                                attn_n[:, g, mc * 128:(mc + 1) * 128],
                                ident[:])
                        nc.vector.tensor_copy(attnT[:, mc, :], pt[:])
                    # out = vT.T @ attnT + (x + bv)
                    for oc in range(ICH):
                        po = psum_tile(oc)
                        for mc in range(4):
                            nc.tensor.matmul(
                                po[:],
                                vT_t[:, mc, oc * 128:(oc + 1) * 128],
                                attnT[:, mc, :],
                                start=(mc == 0), stop=(mc == 3))
                        ot = opool.tile([128, N], F32, tag="ot", name="ot")
                        nc.vector.tensor_add(ot[:], po[:], xr[:, oc, :])
                        nc.sync.dma_start(
                            out[b, oc * 128:(oc + 1) * 128, :], ot[:])
    nc.compile()
    return nc


def _prep_weight(w):
    # [O, I, kd, kh, kw] -> [t, i, o] contiguous
    return np.ascontiguousarray(
        w.transpose(2, 3, 4, 1, 0).reshape(27, C, C)).astype(np.float32)


def kernel(x, y, wq, bq, wk, bk, wv, bv):
    global _CACHED_NC, LAST_RESULTS
    x = np.asarray(x, np.float32)
    y = np.asarray(y, np.float32)

    xf = x.reshape(B, C, 8, 8, 8)
    yf = y.reshape(B, C, 8, 8, 8)
    xpad = np.zeros((B, C, 10, 10, 10), np.float32)
    xpad[:, :, 1:9, 1:9, 1:9] = xf
    ypad = np.zeros((B, C, 10, 10, 10), np.float32)
    ypad[:, :, 1:9, 1:9, 1:9] = yf
    xpad = xpad.reshape(B, C, PAD)
    ypad = ypad.reshape(B, C, PAD)
    xres = x.reshape(B, C, N) + np.asarray(bv, np.float32)[None, :, None]

    wqp = _prep_weight(np.asarray(wq, np.float32))
    wkp = _prep_weight(np.asarray(wk, np.float32))
    wvp = _prep_weight(np.asarray(wv, np.float32))
    bqp = np.ascontiguousarray(
        np.asarray(bq, np.float32).reshape(ICH, 128).T)
    bkp = np.ascontiguousarray(
        np.asarray(bk, np.float32).reshape(ICH, 128).T)

    if _CACHED_NC is None:
        _CACHED_NC = _build()

    in_maps = []
    for i in range(NCORES):
        s = slice(i * BPC, (i + 1) * BPC)
        in_maps.append({
            "xp": xpad[s], "yp": ypad[s], "xres": xres[s],
            "wqp": wqp, "wkp": wkp, "wvp": wvp,
            "bqp": bqp, "bkp": bkp,
        })

    res = run_bass_kernel_spmd(_CACHED_NC, in_maps, list(range(NCORES)))
    LAST_RESULTS = res
    full = np.concatenate([res.results[i]["out"] for i in range(NCORES)],
                          axis=0)
    return full.reshape(B, C, 8, 8, 8)



# revision 2
# speedup vs baseline: 2.2433x; 2.2433x over previous
"""CrossAttention via Winograd F(2,3)^3 convs on 8 TRN2 cores, fp16.

Per-core (data-parallel, 2 batches):
  - Input transform (DVE, fp16): fused H+W stage then D stage; B^T rows are
    {0,+-1} so every output is a short chain of adds/subs of shifted slabs
    (axes pre-factored [5,2] on host so stride-2 taps are unit-stride slices).
  - Conv matmuls (PE, fp16): per comp, stationary = V-slice [128ic x 128(b,t)],
    moving = host-transformed U[comp] [128ic x 512oc] halves; psum [tb, oc]
    accumulates over 8 ic-blocks at 1 cyc/row with LDWEIGHTS hidden.
  - Output transform (DVE): A^T rows are {0,+-1}: psums fold into T1[w] ->
    T2[h,w] -> OUT[d,h,w] accumulators with adds/subs.
  - OUT [tb, pos, oc] is PE-transposed to channel-major [c, n~] (n~ =
    tile-major token order; host pre/post-permutes), conv bias added during
    evacuation (bv folded into the residual), then spilled to DRAM.
  - Attention per batch: scores psum [n~128 x 512] over 8 c-blocks, softmax
    (exp on ScalarE), attn^T + vT via PE transposes, AV, residual add.
"""
import sys

sys.path.insert(0, '/opt/trn_rl_repo')

import numpy as np

from concourse import bacc, mybir, masks
from concourse.tile import TileContext
from concourse.bass_utils import run_bass_kernel_spmd

F16 = mybir.dt.float16
F32 = mybir.dt.float32

B, C, N = 16, 1024, 512
NCORES = 8
BPC = B // NCORES
ICH = C // 128

_CACHED_NC = None
LAST_RESULTS = None

# B^T rows of F(2,3): (sign, f-slice, parity) taps; tap position = 2*tile + d
_ROW = [
    [(1, slice(0, 4), 0), (-1, slice(1, 5), 0)],   # c0 = d0 - d2
    [(1, slice(0, 4), 1), (1, slice(1, 5), 0)],    # c1 = d1 + d2
    [(1, slice(1, 5), 0), (-1, slice(0, 4), 1)],   # c2 = d2 - d1
    [(1, slice(0, 4), 1), (-1, slice(1, 5), 1)],   # c3 = d1 - d3
]


def _acc_terms(nc, dst, terms):
    """dst = sum of signed term APs, via tensor add/sub chains."""
    pos = [t for s, t in terms if s > 0]
    neg = [t for s, t in terms if s < 0]
    assert pos
    if len(pos) >= 2:
        nc.vector.tensor_add(dst, pos[0], pos[1])
        rest_p, rest_n = pos[2:], neg
    else:
        nc.vector.tensor_sub(dst, pos[0], neg[0])
        rest_p, rest_n = [], neg[1:]
    for t in rest_p:
        nc.vector.tensor_add(dst, dst, t)
    for t in rest_n:
        nc.vector.tensor_sub(dst, dst, t)


def _build():
    nc = bacc.Bacc("TRN2", target_bir_lowering=False, debug=False)

    xp = nc.dram_tensor("xp", [BPC, C, 1000], F16, kind="ExternalInput")
    yp = nc.dram_tensor("yp", [BPC, C, 1000], F16, kind="ExternalInput")
    uq = nc.dram_tensor("uq", [64, ICH, 128, C], F16, kind="ExternalInput")
    uk = nc.dram_tensor("uk", [64, ICH, 128, C], F16, kind="ExternalInput")
    uv = nc.dram_tensor("uv", [64, ICH, 128, C], F16, kind="ExternalInput")
    bqp = nc.dram_tensor("bqp", [128, ICH], F32, kind="ExternalInput")
    bkp = nc.dram_tensor("bkp", [128, ICH], F32, kind="ExternalInput")
    xres = nc.dram_tensor("xres", [BPC, C, N], F32, kind="ExternalInput")
    out = nc.dram_tensor("out", [BPC, C, N], F32, kind="ExternalOutput")

    q_dr = nc.dram_tensor("q_dr", [128, ICH, BPC, 64, 8], F16)
    k_dr = nc.dram_tensor("k_dr", [128, ICH, BPC, 64, 8], F16)
    v_dr = nc.dram_tensor("v_dr", [128, ICH, BPC, 64, 8], F16)

    with TileContext(nc) as tc:
        with tc.tile_pool(name="const", bufs=1) as cpool, \
             tc.tile_pool(name="psum", bufs=1, space="PSUM") as psp:

            ident = cpool.tile([128, 128], F16, tag="ident")
            masks.make_identity(nc, ident[:])
            bq_t = cpool.tile([128, ICH], F32, tag="bq_t")
            nc.sync.dma_start(bq_t[:], bqp[:])
            bk_t = cpool.tile([128, ICH], F32, tag="bk_t")
            nc.sync.dma_start(bk_t[:], bkp[:])

            def build_V(V, src_dram, tmp):
                # V [p, icb, cd, ch, cw, b, td, th, tw]
                for b in range(BPC):
                    xpad = tmp.tile([128, ICH, 5, 2, 5, 2, 5, 2], F16,
                                    tag="xpad", name="xpad")
                    nc.sync.dma_start(
                        xpad[:],
                        src_dram[b].rearrange(
                            "(i p) (fd dd fh dh fw dw) -> "
                            "p i fd dd fh dh fw dw",
                            p=128, fd=5, dd=2, fh=5, dh=2, fw=5, dw=2))
                    # fused H+W stage: (h,w) -> (ch,cw,th,tw)
                    VH = tmp.tile([128, ICH, 5, 2, 4, 4, 4, 4], F16,
                                  tag="vh", name="vh")
                    for ch in range(4):
                        for cw in range(4):
                            terms = [
                                (sh * sw,
                                 xpad[:, :, :, :, fh, dh, fw, dw])
                                for sh, fh, dh in _ROW[ch]
                                for sw, fw, dw in _ROW[cw]
                            ]
                            _acc_terms(nc, VH[:, :, :, :, ch, cw, :, :],
                                       terms)
                    # D stage: d -> (cd, td)
                    for cd in range(4):
                        for ch in range(4):
                            for cw in range(4):
                                terms = [
                                    (s, VH[:, :, fd, dd, ch, cw, :, :])
                                    for s, fd, dd in _ROW[cd]
                                ]
                                _acc_terms(
                                    nc, V[:, :, cd, ch, cw, b, :, :, :],
                                    terms)

            def conv(V, u_dram, bias_t, dst_dr, cpool_, upool):
                T1 = cpool_.tile([128, 2, C], F16, tag="t1", name="t1")
                T2 = cpool_.tile([128, 2, 2, C], F16, tag="t2", name="t2")
                OUT = cpool_.tile([128, 2, 2, 2, C], F16, tag="oacc",
                                  name="oacc")
                for comp in range(64):
                    cd, ch, cw = comp // 16, (comp // 4) % 4, comp % 4
                    ut = upool.tile([128, ICH, C], F16, tag="ut", name="ut")
                    nc.sync.dma_start(
                        ut[:], u_dram[comp].rearrange("i p o -> p i o"))
                    pslo = psp.tile([128, 512], F32, tag=f"pslo{comp % 2}",
                                    name=f"pslo{comp % 2}")
                    pshi = psp.tile([128, 512], F32, tag=f"pshi{comp % 2}",
                                    name=f"pshi{comp % 2}")
                    for icb in range(ICH):
                        lhsT = V[:, icb, cd, ch, cw, :, :, :, :]
                        nc.tensor.matmul(pslo[:], lhsT, ut[:, icb, 0:512],
                                         start=(icb == 0),
                                         stop=(icb == ICH - 1))
                        nc.tensor.matmul(pshi[:], lhsT, ut[:, icb, 512:1024],
                                         start=(icb == 0),
                                         stop=(icb == ICH - 1))
                    for half, ps in ((0, pslo), (1, pshi)):
                        sl = slice(half * 512, half * 512 + 512)
                        if cw == 0:
                            nc.vector.tensor_copy(T1[:, 0, sl], ps[:])
                        elif cw == 1:
                            nc.vector.tensor_add(T1[:, 0, sl], T1[:, 0, sl],
                                                 ps[:])
                            nc.vector.tensor_copy(T1[:, 1, sl], ps[:])
                        elif cw == 2:
                            nc.vector.tensor_add(T1[:, 0, sl], T1[:, 0, sl],
                                                 ps[:])
                            nc.vector.tensor_sub(T1[:, 1, sl], T1[:, 1, sl],
                                                 ps[:])
                        else:
                            nc.vector.tensor_sub(T1[:, 1, sl], T1[:, 1, sl],
                                                 ps[:])
                    if cw == 3:
                        if ch == 0:
                            nc.vector.tensor_copy(T2[:, 0], T1[:])
                        elif ch == 1:
                            nc.vector.tensor_add(T2[:, 0], T2[:, 0], T1[:])
                            nc.vector.tensor_copy(T2[:, 1], T1[:])
                        elif ch == 2:
                            nc.vector.tensor_add(T2[:, 0], T2[:, 0], T1[:])
                            nc.vector.tensor_sub(T2[:, 1], T2[:, 1], T1[:])
                        else:
                            nc.vector.tensor_sub(T2[:, 1], T2[:, 1], T1[:])
                    if cw == 3 and ch == 3:
                        if cd == 0:
                            nc.vector.tensor_copy(OUT[:, 0], T2[:])
                        elif cd == 1:
                            nc.vector.tensor_add(OUT[:, 0], OUT[:, 0],
                                                 T2[:])
                            nc.vector.tensor_copy(OUT[:, 1], T2[:])
                        elif cd == 2:
                            nc.vector.tensor_add(OUT[:, 0], OUT[:, 0],
                                                 T2[:])
                            nc.vector.tensor_sub(OUT[:, 1], OUT[:, 1],
                                                 T2[:])
                        else:
                            nc.vector.tensor_sub(OUT[:, 1], OUT[:, 1],
                                                 T2[:])
                # transpose to channel-major and spill
                qc = cpool_.tile([128, ICH, BPC, 64, 8], F16, tag="qc",
                                 name="qc")
                for p8 in range(8):
                    pd, ph, pw = p8 // 4, (p8 // 2) % 2, p8 % 2
                    for ocb in range(ICH):
                        ptr = psp.tile([128, BPC, 64], F16, tag="ptr",
                                       name="ptr")
                        nc.tensor.transpose(
                            ptr[:].rearrange("p b t -> p (b t)"),
                            OUT[:, pd, ph, pw, ocb * 128:(ocb + 1) * 128],
                            ident[:])
                        dst = qc[:, ocb, :, :, p8]
                        if bias_t is not None:
                            nc.vector.tensor_scalar_add(
                                dst, ptr[:], bias_t[:, ocb:ocb + 1])
                        else:
                            nc.vector.tensor_copy(dst, ptr[:])
                nc.sync.dma_start(dst_dr[:], qc[:])

            # ---------------- conv section ----------------
            with tc.tile_pool(name="vbuf", bufs=1) as vpool:
                V = vpool.tile([128, ICH, 4, 4, 4, BPC, 4, 4, 4], F16,
                               tag="V", name="V")
                with tc.tile_pool(name="tmp", bufs=1) as tmp:
                    build_V(V, xp, tmp)
                with tc.tile_pool(name="cq", bufs=1) as cp, \
                     tc.tile_pool(name="uq_p", bufs=2) as up:
                    conv(V, uq, bq_t, q_dr, cp, up)
                with tc.tile_pool(name="tmp2", bufs=1) as tmp:
                    build_V(V, yp, tmp)
                with tc.tile_pool(name="ck", bufs=1) as cp, \
                     tc.tile_pool(name="uk_p", bufs=2) as up:
                    conv(V, uk, bk_t, k_dr, cp, up)
                with tc.tile_pool(name="cv", bufs=1) as cp, \
                     tc.tile_pool(name="uv_p", bufs=2) as up:
                    conv(V, uv, None, v_dr, cp, up)

            # ---------------- attention ----------------
            with tc.tile_pool(name="attn", bufs=1) as ap:
                vc_t = ap.tile([128, ICH, BPC, 64, 8], F16, tag="vct",
                               name="vct")
                nc.sync.dma_start(vc_t[:], v_dr[:])
                vT = ap.tile([128, 4, BPC, C], F16, tag="vT", name="vT")
                for b in range(BPC):
                    for cb in range(ICH):
                        ptv = psp.tile([128, 4, 128], F16, tag="ptr4",
                                       name="ptr4")
                        for blk in range(4):
                            nc.tensor.transpose(
                                ptv[:, blk, :],
                                vc_t[:, cb, b, :, :].rearrange(
                                    "p t e -> p (t e)")[
                                    :, blk * 128:(blk + 1) * 128],
                                ident[:])
                        nc.vector.tensor_copy(
                            vT[:, :, b, cb * 128:(cb + 1) * 128], ptv[:])

                for b in range(BPC):
                    qc_t = ap.tile([128, ICH, 64, 8], F16, tag="qct",
                                   name="qct")
                    nc.sync.dma_start(qc_t[:], q_dr[:, :, b])
                    kc_t = ap.tile([128, ICH, 64, 8], F16, tag="kct",
                                   name="kct")
                    nc.sync.dma_start(kc_t[:], k_dr[:, :, b])
                    xr = ap.tile([128, ICH, N], F32, tag="xr", name="xr")
                    nc.sync.dma_start(
                        xr[:], xres[b].rearrange("(i p) n -> p i n", p=128))

                    stats = ap.tile([128, 3, 4], F32, tag="stats",
                                    name="stats")
                    attn_n = ap.tile([128, 4, N], F16, tag="attn_n",
                                     name="attn_n")
                    for g in range(4):
                        ps = psp.tile([128, 512], F32, tag="ps_s",
                                      name="ps_s")
                        for cb in range(ICH):
                            nc.tensor.matmul(
                                ps[:],
                                qc_t[:, cb, g * 16:(g + 1) * 16, :],
                                kc_t[:, cb].rearrange("p t e -> p (t e)"),
                                start=(cb == 0), stop=(cb == ICH - 1))
                        negmax = stats[:, 0, g:g + 1]
                        esum = stats[:, 1, g:g + 1]
                        rinv = stats[:, 2, g:g + 1]
                        nc.vector.reduce_max(negmax, ps[:],
                                             axis=mybir.AxisListType.X,
                                             negate=True)
                        nc.scalar.activation(
                            attn_n[:, g, :], ps[:],
                            mybir.ActivationFunctionType.Exp,
                            bias=negmax, accum_out=esum)
                        nc.vector.reciprocal(rinv, esum)
                        nc.vector.tensor_scalar_mul(attn_n[:, g, :],
                                                    attn_n[:, g, :], rinv)
                    attnT = ap.tile([128, 4, N], F16, tag="attnT",
                                    name="attnT")
                    for mc in range(4):
                        ptv = psp.tile([128, 4, 128], F16, tag="ptr4",
                                       name="ptr4")
                        for g in range(4):
                            nc.tensor.transpose(
                                ptv[:, g, :],
                                attn_n[:, g, mc * 128:(mc + 1) * 128],
                                ident[:])
                        nc.vector.tensor_copy(
                            attnT[:, mc, :],
                            ptv[:].rearrange("p g n -> p (g n)"))
                    for cb in range(ICH):
                        po = psp.tile([128, 512], F32, tag="ps_o",
                                      name="ps_o")
                        for mc in range(4):
                            nc.tensor.matmul(
                                po[:],
                                vT[:, mc, b, cb * 128:(cb + 1) * 128],
                                attnT[:, mc, :],
                                start=(mc == 0), stop=(mc == 3))
                        ot = ap.tile([128, N], F32, tag="ot", name="ot")
                        nc.vector.tensor_add(ot[:], po[:], xr[:, cb, :])
                        nc.sync.dma_start(
                            out[b, cb * 128:(cb + 1) * 128, :], ot[:])
    nc.compile()
    return nc


# ---------------- host-side prep ----------------
_G = np.array([[1, 0, 0], [.5, .5, .5], [.5, -.5, .5], [0, 0, 1]], np.float32)


def _prep_U(w):
    # w [O, I, kd, kh, kw] -> U [64(comp), ICH, 128(ic%), O] fp16
    w6 = np.asarray(w, np.float32)
    U = np.einsum('ad,be,cf,oidef->abcio', _G, _G, _G, w6, optimize=True)
    return np.ascontiguousarray(
        U.reshape(64, ICH, 128, C)).astype(np.float16)


def _tile_perm(a):
    # [.., C, 8,8,8] -> n~ = (td,th,tw,pd,ph,pw) token order
    s = a.shape[:-3]
    a = a.reshape(*s, 4, 2, 4, 2, 4, 2)
    nd = len(s)
    a = a.transpose(*range(nd), nd, nd + 2, nd + 4, nd + 1, nd + 3, nd + 5)
    return a.reshape(*s, 512)


def _tile_unperm(a):
    s = a.shape[:-1]
    a = a.reshape(*s, 4, 4, 4, 2, 2, 2)
    nd = len(s)
    a = a.transpose(*range(nd), nd, nd + 3, nd + 1, nd + 4, nd + 2, nd + 5)
    return a.reshape(*s, 8, 8, 8)


def _host_prep(x, y, wq, bq, wk, bk, wv, bv):
    x = np.asarray(x, np.float32)
    y = np.asarray(y, np.float32)

    def padded(a):
        ap_ = np.zeros((B, C, 10, 10, 10), np.float16)
        ap_[:, :, 1:9, 1:9, 1:9] = a.reshape(B, C, 8, 8, 8)
        # factor each axis [10] -> [5,2]
        ap_ = ap_.reshape(B, C, 5, 2, 5, 2, 5, 2)
        return np.ascontiguousarray(ap_).reshape(B, C, 1000)

    xpad = padded(x)
    ypad = padded(y)
    xres = _tile_perm(x.reshape(B, C, 8, 8, 8)) \
        + np.asarray(bv, np.float32)[None, :, None]
    uq_h = _prep_U(wq)
    uk_h = _prep_U(wk)
    uv_h = _prep_U(wv)
    bqp = np.ascontiguousarray(np.asarray(bq, np.float32).reshape(ICH, 128).T)
    bkp = np.ascontiguousarray(np.asarray(bk, np.float32).reshape(ICH, 128).T)
    return xpad, ypad, xres, uq_h, uk_h, uv_h, bqp, bkp


def kernel(x, y, wq, bq, wk, bk, wv, bv):
    global _CACHED_NC, LAST_RESULTS
    xpad, ypad, xres, uq_h, uk_h, uv_h, bqp, bkp = _host_prep(
        x, y, wq, bq, wk, bk, wv, bv)

    if _CACHED_NC is None:
        _CACHED_NC = _build()

    in_maps = []
    for i in range(NCORES):
        s = slice(i * BPC, (i + 1) * BPC)
        in_maps.append({
            "xp": xpad[s], "yp": ypad[s], "xres": xres[s],
            "uq": uq_h, "uk": uk_h, "uv": uv_h,
            "bqp": bqp, "bkp": bkp,
        })

    res = run_bass_kernel_spmd(_CACHED_NC, in_maps, list(range(NCORES)))
    LAST_RESULTS = res
    full = np.concatenate([res.results[i]["out"] for i in range(NCORES)],
                          axis=0)
    return _tile_unperm(full).astype(np.float32)


# revision 3
# speedup vs baseline: 2.4525x; 1.0933x over previous
"""CrossAttention via Winograd F(2,3)^3, comp+batch hybrid sharding on 8
TRN2 cores, fp16, with a ReduceScatter before attention.

Sharding: core c -> bg = c//2 (batches [4bg, 4bg+4)), cdp = c%2 (d-component
pair: cd in {2cdp, 2cdp+1}). Each core computes 32 of the 64 Winograd
components (its 2 cd, all ch, all cw) for its 4 batches, halving the U-weight
stream (201MB vs 402MB/core). The d-axis transforms are the only per-core
difference and are expressed as coefficient MACs (coeffs from in_maps), so
the SPMD program is identical on all cores:
  - input D-stage: XD[cdl] = sum_j Bt[2cdp+cdl, j] * dtap_j  (on the small
    pre-expansion xpad; W/H stages structural {0,+-1} adds)
  - output d-apply: OUT[d] += At[d, 2cdp+cdl] * T2[cdl]  (h/w folds
    structural)
A 2-core ReduceScatter sums the cd-pair partial outputs and hands each core
its 2 attention batches; transposes+bias then per-batch attention as in the
data-parallel version.
"""
import sys

sys.path.insert(0, '/opt/trn_rl_repo')

import numpy as np

from concourse import bacc, mybir, masks
from concourse.tile import TileContext
from concourse.bass_utils import run_bass_kernel_spmd

F16 = mybir.dt.float16
F32 = mybir.dt.float32

B, C, N = 16, 1024, 512
NCORES = 8
ICH = C // 128
BG = 4        # batches per batch-group (per-core conv work)
BPC = 2       # attention batches per core
NJ = 2        # tb chunks of 128 = batch pairs = RS shards

_CACHED_NC = None
LAST_RESULTS = None

# B^T rows of F(2,3): (sign, f-slice, parity); tap position = 2*tile + d
_ROW = [
    [(1, slice(0, 4), 0), (-1, slice(1, 5), 0)],   # c0 = d0 - d2
    [(1, slice(0, 4), 1), (1, slice(1, 5), 0)],    # c1 = d1 + d2
    [(1, slice(1, 5), 0), (-1, slice(0, 4), 1)],   # c2 = d2 - d1
    [(1, slice(0, 4), 1), (-1, slice(1, 5), 1)],   # c3 = d1 - d3
]
_TAP = [(slice(0, 4), 0), (slice(0, 4), 1), (slice(1, 5), 0),
        (slice(1, 5), 1)]

_BT = np.array([[1, 0, -1, 0], [0, 1, 1, 0], [0, -1, 1, 0], [0, 1, 0, -1]],
               np.float32)
_AT = np.array([[1, 1, 1, 0], [0, 1, -1, -1]], np.float32)
_G = np.array([[1, 0, 0], [.5, .5, .5], [.5, -.5, .5], [0, 0, 1]], np.float32)

_MULT = mybir.AluOpType.mult
_ADD = mybir.AluOpType.add


def _acc_terms(eng, dst, terms):
    pos = [t for s, t in terms if s > 0]
    neg = [t for s, t in terms if s < 0]
    assert pos
    if len(pos) >= 2:
        eng.tensor_add(dst, pos[0], pos[1])
        rest_p, rest_n = pos[2:], neg
    else:
        eng.tensor_sub(dst, pos[0], neg[0])
        rest_p, rest_n = [], neg[1:]
    for t in rest_p:
        eng.tensor_add(dst, dst, t)
    for t in rest_n:
        eng.tensor_sub(dst, dst, t)


def _build(skip_rs=False):
    nc = bacc.Bacc("TRN2", target_bir_lowering=False, debug=False,
                   num_devices=NCORES)

    xp = nc.dram_tensor("xp", [BG, 128, 10, 10, 10, ICH], F16,
                        kind="ExternalInput")
    yp = nc.dram_tensor("yp", [BG, 128, 10, 10, 10, ICH], F16,
                        kind="ExternalInput")
    uq = nc.dram_tensor("uq", [32, ICH, 128, C], F16, kind="ExternalInput")
    uk = nc.dram_tensor("uk", [32, ICH, 128, C], F16, kind="ExternalInput")
    uv = nc.dram_tensor("uv", [32, ICH, 128, C], F16, kind="ExternalInput")
    bqx = nc.dram_tensor("bqx", [128, 2, 512], F16, kind="ExternalInput")
    bkx = nc.dram_tensor("bkx", [128, 2, 512], F16, kind="ExternalInput")
    # cols 0..7: in-T D-MAC c[cdl*4+j]; cols 8..11: d-apply s[cdl*2+d]
    coefs = nc.dram_tensor("coefs", [128, 12], F32, kind="ExternalInput")
    xres = nc.dram_tensor("xres", [BPC, C, N], F32, kind="ExternalInput")
    out = nc.dram_tensor("out", [BPC, C, N], F32, kind="ExternalOutput")

    # RS spill: [conv, shard_j, tb128, pos8, half, 512]
    sp = nc.dram_tensor("sp", [3, NJ, 128, 8, 2, 512], F16)
    red_b = nc.dram_tensor("red_b", [3, 128, 8, 2, 512], F16)

    with TileContext(nc) as tc:
        with tc.tile_pool(name="const", bufs=1) as cpool, \
             tc.tile_pool(name="psum", bufs=1, space="PSUM") as psp:

            ident = cpool.tile([128, 128], F16, tag="ident")
            masks.make_identity(nc, ident[:])
            bq_t = cpool.tile([128, 2, 512], F16, tag="bq_t")
            nc.sync.dma_start(bq_t[:], bqx[:])
            bk_t = cpool.tile([128, 2, 512], F16, tag="bk_t")
            nc.sync.dma_start(bk_t[:], bkx[:])
            cf = cpool.tile([128, 12], F32, tag="cf")
            nc.sync.dma_start(cf[:], coefs[:])

            def build_V(V, src_dram, tmp):
                # V [p, cdl, ch, cw, b, td, th, tw, icb]  (icb innermost so
                # (tw icb) merges keep every AP within 5 dims)
                for b in range(BG):
                    xpad = tmp.tile([128, 5, 2, 5, 2, 5, 2, ICH], F16,
                                    tag="xpad", name="xpad")
                    nc.sync.dma_start(
                        xpad[:],
                        src_dram[b].rearrange(
                            "p (fd dd) (fh dh) (fw dw) i -> "
                            "p fd dd fh dh fw dw i",
                            fd=5, fh=5, fw=5))
                    # D stage (per-core coefficient MACs; 3D APs)
                    XD = tmp.tile([128, 2, 4, 5, 2, 5, 2, ICH], F16,
                                  tag="xd", name="xd")
                    for cdl in range(2):
                        eng = nc.vector
                        for j in range(4):
                            fs, d = _TAP[j]
                            tap = xpad[:, fs, d].rearrange(
                                "p fd fh dh fw dw i -> p fd (fh dh fw dw i)")
                            sc = cf[:, cdl * 4 + j:cdl * 4 + j + 1]
                            dst = XD[:, cdl].rearrange(
                                "p td fh dh fw dw i -> p td (fh dh fw dw i)")
                            if j == 0:
                                eng.tensor_scalar_mul(dst, tap, sc)
                            else:
                                eng.scalar_tensor_tensor(
                                    dst, tap, sc, dst, _MULT, _ADD)
                    for cdl in range(2):
                        w_eng = nc.vector if cdl == 0 else nc.gpsimd
                        # W stage (structural)
                        XW = tmp.tile([128, 4, 5, 2, 4, 4, ICH], F16,
                                      tag="xw", name="xw", bufs=2)
                        for cw in range(4):
                            terms = [
                                (s, XD[:, cdl, :, :, :, fs, d, :].rearrange(
                                    "p td fh dh fw i -> p td (fh dh) fw i"))
                                for s, fs, d in _ROW[cw]]
                            dstw = XW[:, :, :, :, cw, :, :].rearrange(
                                "p td fh dh tw i -> p td (fh dh) tw i")
                            _acc_terms(w_eng, dstw, terms)
                        # H stage (structural, DVE)
                        for ch in range(4):
                            terms = [
                                (s, XW[:, :, fs, d, :, :, :].rearrange(
                                    "p td th cw tw i -> p td th cw (tw i)"))
                                for s, fs, d in _ROW[ch]]
                            dstv = V[:, cdl, ch, :, b, :, :, :, :]
                            dstv = dstv.rearrange(
                                "p cw td th tw i -> p td th cw (tw i)")
                            _acc_terms(nc.vector, dstv, terms)

            def conv(V, u_dram, conv_i, cp, upool, psc):
                mm = 0
                for half in range(2):
                    osl = slice(half * 512, half * 512 + 512)
                    OUT = cp.tile([128, NJ, 2, 2, 2, 512], F16, tag="oacc",
                                  name="oacc", bufs=2)
                    for cdl in range(2):
                        T2 = cp.tile([128, NJ, 2, 2, 512], F16, tag="t2",
                                     name="t2")
                        for ch in range(4):
                            T1 = cp.tile([128, NJ, 2, 512], F16, tag="t1",
                                         name="t1", bufs=2)
                            for cw in range(4):
                                comp = (cdl * 4 + ch) * 4 + cw
                                ut = upool.tile([128, ICH, 512], F16,
                                                tag="ut", name="ut")
                                nc.scalar.dma_start(
                                    ut[:],
                                    u_dram[comp, :, :, osl].rearrange(
                                        "i p o -> p i o"))
                                for j in range(NJ):
                                    ps = psc.tile([128, 512], F32,
                                                  tag=f"ps{mm % 6}",
                                                  name=f"ps{mm % 6}")
                                    mm += 1
                                    for icb in range(ICH):
                                        lhsT = V[:, cdl, ch, cw,
                                                 2 * j:2 * j + 2,
                                                 :, :, :, icb]
                                        nc.tensor.matmul(
                                            ps[:], lhsT, ut[:, icb, :],
                                            start=(icb == 0),
                                            stop=(icb == ICH - 1))
                                    # w fold (structural)
                                    t0 = T1[:, j, 0, :]
                                    t1 = T1[:, j, 1, :]
                                    if cw == 0:
                                        nc.scalar.copy(t0, ps[:])
                                    elif cw == 1:
                                        nc.vector.tensor_add(t0, t0, ps[:])
                                        nc.scalar.copy(t1, ps[:])
                                    elif cw == 2:
                                        nc.vector.tensor_add(t0, t0, ps[:])
                                        nc.vector.tensor_sub(t1, t1, ps[:])
                                    else:
                                        nc.vector.tensor_sub(t1, t1, ps[:])
                            # h fold (structural), both j in one op
                            src = T1[:]
                            h0 = T2[:, :, 0]
                            h1 = T2[:, :, 1]
                            if ch == 0:
                                nc.scalar.copy(h0, src)
                            elif ch == 1:
                                nc.vector.tensor_add(h0, h0, src)
                                nc.scalar.copy(h1, src)
                            elif ch == 2:
                                nc.vector.tensor_add(h0, h0, src)
                                nc.vector.tensor_sub(h1, h1, src)
                            else:
                                nc.vector.tensor_sub(h1, h1, src)
                        # d apply (per-core coefs), both j in one op
                        for d in range(2):
                            sc = cf[:, 8 + cdl * 2 + d:9 + cdl * 2 + d]
                            dst = OUT[:, :, d]
                            src = T2[:]
                            if cdl == 0:
                                nc.vector.tensor_scalar_mul(dst, src, sc)
                            else:
                                nc.vector.scalar_tensor_tensor(
                                    dst, src, sc, dst, _MULT, _ADD)
                    for j in range(NJ):
                        nc.gpsimd.dma_start(
                            sp[conv_i, j, :, :, half, :],
                            OUT[:, j].rearrange("p d h w o -> p (d h w) o"))

            # ---------------- conv section ----------------
            with tc.tile_pool(name="vbuf", bufs=1) as vpool, \
                 tc.tile_pool(name="uw", bufs=3) as upool, \
                 tc.tile_pool(name="psumc", bufs=1, space="PSUM") as psc:
                V = vpool.tile([128, 2, 4, 4, BG, 4, 4, 4, ICH], F16,
                               tag="V", name="V")
                with tc.tile_pool(name="tmp", bufs=1) as tmp:
                    build_V(V, xp, tmp)
                def rs_conv(ci):
                    if skip_rs:
                        nc.sync.dma_start(red_b[ci], sp[ci, 0])
                    else:
                        nc.gpsimd.collective_compute(
                            "ReduceScatter",
                            mybir.AluOpType.add,
                            replica_groups=[[0, 1], [2, 3], [4, 5], [6, 7]],
                            ins=[sp[ci]],
                            outs=[red_b[ci]],
                        )

                with tc.tile_pool(name="cq", bufs=1) as cp:
                    conv(V, uq, 0, cp, upool, psc)
                rs_conv(0)
                with tc.tile_pool(name="tmp2", bufs=1) as tmp:
                    build_V(V, yp, tmp)
                with tc.tile_pool(name="ck", bufs=1) as cp:
                    conv(V, uk, 1, cp, upool, psc)
                rs_conv(1)
                with tc.tile_pool(name="cv", bufs=1) as cp:
                    conv(V, uv, 2, cp, upool, psc)
                rs_conv(2)

            # ---------------- attention ----------------
            with tc.tile_pool(name="attn", bufs=1) as ap, \
                 tc.tile_pool(name="psuma", bufs=1, space="PSUM") as psa:
                red = ap.tile([128, 3, 8, 2, 512], F16, tag="red",
                              name="red")
                nc.sync.dma_start(
                    red[:], red_b[:].rearrange("c p e h o -> p c e h o"))

                # conv bias: add to red in token-major form (bias rows are
                # replicated across partitions by the host)
                for ci, bias in ((0, bq_t), (1, bk_t)):
                    for p8 in range(8):
                        nc.vector.tensor_add(red[:, ci, p8],
                                             red[:, ci, p8], bias[:])
                qkv = []
                for ci in range(3):
                    qc = ap.tile([128, ICH, BPC, 64, 8], F16,
                                 tag=f"qc{ci}", name=f"qc{ci}")
                    for p8 in range(8):
                        for g4 in range(2):
                            ptr = psp.tile([128, 4, 128], F16, tag="ptr4",
                                           name="ptr4", bufs=2)
                            for k in range(4):
                                nc.tensor.transpose(
                                    ptr[:, k, :],
                                    red[:, ci, p8, g4,
                                        k * 128:(k + 1) * 128],
                                    ident[:])
                            nc.vector.tensor_copy(
                                qc[:, g4 * 4:(g4 + 1) * 4, :, :, p8],
                                ptr[:].rearrange("p k (b t) -> p k b t",
                                                 b=BPC))
                    qkv.append(qc)
                qc_t, kc_t, vc_t = qkv

                vT = ap.tile([128, 4, BPC, C], F16, tag="vT", name="vT")
                for b in range(BPC):
                    for cb in range(ICH):
                        ptv = psp.tile([128, 4, 128], F16, tag="ptr4",
                                       name="ptr4", bufs=2)
                        for blk in range(4):
                            nc.tensor.transpose(
                                ptv[:, blk, :],
                                vc_t[:, cb, b, :, :].rearrange(
                                    "p t e -> p (t e)")[
                                    :, blk * 128:(blk + 1) * 128],
                                ident[:])
                        nc.vector.tensor_copy(
                            vT[:, :, b, cb * 128:(cb + 1) * 128], ptv[:])

                for b in range(BPC):
                    xr = ap.tile([128, ICH, N], F32, tag=f"xr{b}",
                                 name=f"xr{b}")
                    nc.sync.dma_start(
                        xr[:], xres[b].rearrange("(i p) n -> p i n", p=128))

                    stats = ap.tile([128, 3, 4], F32, tag=f"stats{b}",
                                    name=f"stats{b}")
                    attn_n = ap.tile([128, 4, N], F16, tag=f"attn_n{b}",
                                     name=f"attn_n{b}")
                    for g in range(4):
                        ps = psa.tile([128, 512], F32, tag=f"ps_s{b}",
                                      name=f"ps_s{b}")
                        for cb in range(ICH):
                            nc.tensor.matmul(
                                ps[:],
                                qc_t[:, cb, b, g * 16:(g + 1) * 16, :],
                                kc_t[:, cb, b].rearrange(
                                    "p t e -> p (t e)"),
                                start=(cb == 0), stop=(cb == ICH - 1))
                        negmax = stats[:, 0, g:g + 1]
                        esum = stats[:, 1, g:g + 1]
                        rinv = stats[:, 2, g:g + 1]
                        nc.vector.reduce_max(negmax, ps[:],
                                             axis=mybir.AxisListType.X,
                                             negate=True)
                        nc.scalar.activation(
                            attn_n[:, g, :], ps[:],
                            mybir.ActivationFunctionType.Exp,
                            bias=negmax, accum_out=esum)
                        nc.vector.reciprocal(rinv, esum)
                        nc.vector.tensor_scalar_mul(attn_n[:, g, :],
                                                    attn_n[:, g, :], rinv)
                    attnT = ap.tile([128, 4, N], F16, tag=f"attnT{b}",
                                    name=f"attnT{b}")
                    for mc in range(4):
                        ptv = psp.tile([128, 4, 128], F16, tag="ptr4",
                                       name="ptr4", bufs=2)
                        for g in range(4):
                            nc.tensor.transpose(
                                ptv[:, g, :],
                                attn_n[:, g, mc * 128:(mc + 1) * 128],
                                ident[:])
                        nc.vector.tensor_copy(
                            attnT[:, mc, :],
                            ptv[:].rearrange("p g n -> p (g n)"))
                    for cb in range(ICH):
                        po = psa.tile([128, 512], F32, tag=f"ps_o{b}",
                                      name=f"ps_o{b}")
                        for mc in range(4):
                            nc.tensor.matmul(
                                po[:],
                                vT[:, mc, b, cb * 128:(cb + 1) * 128],
                                attnT[:, mc, :],
                                start=(mc == 0), stop=(mc == 3))
                        ot = ap.tile([128, N], F32, tag=f"ot{b}",
                                     name=f"ot{b}", bufs=2)
                        nc.vector.tensor_add(ot[:], po[:], xr[:, cb, :])
                        nc.sync.dma_start(
                            out[b, cb * 128:(cb + 1) * 128, :], ot[:])
    nc.compile()
    return nc


# ---------------- host-side prep ----------------
_K64 = np.einsum('ad,be,cf->abcdef', _G, _G, _G).reshape(64, 27)


def _prep_U(w):
    # [O, I, 27] @ K.T -> [O*I, 64] -> [64, I(=ICH,128), O]
    w2 = np.asarray(w, np.float32).reshape(C * C, 27)
    P = (w2 @ _K64.T).astype(np.float16)          # [o*i, 64]
    P = P.reshape(C, C, 64).transpose(2, 1, 0)    # [64, i, o]
    return np.ascontiguousarray(P).reshape(64, ICH, 128, C)


def _tile_perm(a):
    s = a.shape[:-3]
    a = a.reshape(*s, 4, 2, 4, 2, 4, 2)
    nd = len(s)
    a = a.transpose(*range(nd), nd, nd + 2, nd + 4, nd + 1, nd + 3, nd + 5)
    return a.reshape(*s, 512)


def _tile_unperm(a):
    s = a.shape[:-1]
    a = a.reshape(*s, 4, 4, 4, 2, 2, 2)
    nd = len(s)
    a = a.transpose(*range(nd), nd, nd + 3, nd + 1, nd + 4, nd + 2, nd + 5)
    return a.reshape(*s, 8, 8, 8)


def _host_prep(x, y, wq, bq, wk, bk, wv, bv):
    x = np.asarray(x, np.float32)
    y = np.asarray(y, np.float32)

    def padded(a):
        ap_ = np.zeros((B, C, 10, 10, 10), np.float16)
        ap_[:, :, 1:9, 1:9, 1:9] = a.reshape(B, C, 8, 8, 8)
        # -> [b, p, d10, h10, w10, icb]
        ap_ = ap_.reshape(B, ICH, 128, 10, 10, 10).transpose(
            0, 2, 3, 4, 5, 1)
        return np.ascontiguousarray(ap_)

    xpad = padded(x)
    ypad = padded(y)
    xres = _tile_perm(x.reshape(B, C, 8, 8, 8)) \
        + np.asarray(bv, np.float32)[None, :, None]
    us = [_prep_U(w) for w in (wq, wk, wv)]
    bqx = np.ascontiguousarray(np.broadcast_to(
        np.asarray(bq, np.float16).reshape(1, 2, 512), (128, 2, 512)))
    bkx = np.ascontiguousarray(np.broadcast_to(
        np.asarray(bk, np.float16).reshape(1, 2, 512), (128, 2, 512)))
    return xpad, ypad, xres, us, bqx, bkx


def _core_in_map(core, xpad, ypad, xres, us, bqx, bkx):
    bg, cdp = core // 2, core % 2
    bs = slice(bg * BG, bg * BG + BG)
    ab = bg * BG + 2 * cdp
    asl = slice(ab, ab + 2)
    sel = [(2 * cdp + cdl) * 16 + ch * 4 + cw
           for cdl in range(2) for ch in range(4) for cw in range(4)]
    coefs = np.zeros((128, 12), np.float32)
    for cdl in range(2):
        for j in range(4):
            coefs[:, cdl * 4 + j] = _BT[2 * cdp + cdl, j]
        for d in range(2):
            coefs[:, 8 + cdl * 2 + d] = _AT[d, 2 * cdp + cdl]
    return {
        "xp": xpad[bs], "yp": ypad[bs], "xres": xres[asl],
        "uq": us[0][sel], "uk": us[1][sel], "uv": us[2][sel],
        "bqx": bqx, "bkx": bkx, "coefs": coefs,
    }


def kernel(x, y, wq, bq, wk, bk, wv, bv):
    global _CACHED_NC, LAST_RESULTS
    xpad, ypad, xres, us, bqx, bkx = _host_prep(
        x, y, wq, bq, wk, bk, wv, bv)

    if _CACHED_NC is None:
        _CACHED_NC = _build()

    in_maps = [_core_in_map(i, xpad, ypad, xres, us, bqx, bkx)
               for i in range(NCORES)]

    res = run_bass_kernel_spmd(_CACHED_NC, in_maps, list(range(NCORES)))
    LAST_RESULTS = res
    # core c holds batches [4*(c//2) + 2*(c%2), +2)
    full = np.empty((B, C, N), np.float32)
    for i in range(NCORES):
        ab = (i // 2) * BG + 2 * (i % 2)
        full[ab:ab + 2] = res.results[i]["out"]
    return _tile_unperm(full).astype(np.float32)
